# revision 1
# baseline (speedup 1.0000x reference)
"""Distributed masked-attention kernel for 8 TRN2 NeuronCores (v3).

Reference computation (B=2, L=1024, D=1024, H=16, DH=64):
    Qz, Kz = masked Q, K;  Qp/Kp/Vp = projections (V = K)
    per-head attention with outer-product validity mask, softmax scaled
    by 1/sqrt(D);  O = Qp + attn;  out = O + relu(mask_q(O @ Wo.T))

Sharding: core c = 2*g + b handles batch b = c%2, head group g = c//2
(4 heads, feature block e = [256g, 256g+256)).  Activations are
feature-major ("X.T" = [features, tokens]) so the TensorEngine contracts
along partitions without transposes; the host pre-transposes Q, K and
the weight shards and casts to bf16 (PSUM accumulation stays f32).

Softmax: scores are small (|S/32| < ~2) so no max-subtraction; exp(S/32 +
bias_k) with bias_k = -30000 at masked k gives exact zeros; the
denominator comes from a ones-column appended to V (M=65 matmul) plus a
1e30 PSUM prefill at masked-q columns so 1/denom ~ 0 there.

Attention runs q-chunk-major (512 tokens per chunk, head pairs packed
into PE row groups, exp pair-fused over 2 PSUM banks).  After each
q-chunk an 8-core AllToAll (64-token chunks per receiver, 0.5MB)
redistributes O.T; the first collective overlaps the second q-chunk.
Each core ends up with 64-token blocks of both batches and both q-halves
for the output-projection epilogue (two passes, one per collective).
The host reassembles the final [2,1024,1024].
"""
import numpy as np

B, L, D = 2, 1024, 1024
H, DH = 16, 64
NCORES = 8
HPC = 4          # heads per core
EB = 256         # feature block per core
NEG = -30000.0   # masked-k bias (exp -> exact 0)
BIG = 1e30       # masked-q denominator prefill

TRACE = False
TRACE_KWARGS = {}
LAST_RESULTS = None

_compiled = None


def _build():
    import concourse.bacc as bacc
    import concourse.tile as tile
    from concourse.tile import add_dep_helper
    from concourse import mybir

    f32 = mybir.dt.float32
    bf16 = mybir.dt.bfloat16
    EXP = mybir.ActivationFunctionType.Exp
    RELU = mybir.ActivationFunctionType.Relu

    nc = bacc.Bacc("TRN2", target_bir_lowering=False, debug=False,
                   num_devices=NCORES)

    qt = nc.dram_tensor("qt", [D, L], bf16, kind="ExternalInput")
    kt = nc.dram_tensor("kt", [D, L], bf16, kind="ExternalInput")
    wq = nc.dram_tensor("wq", [D, EB], bf16, kind="ExternalInput")
    wk = nc.dram_tensor("wk", [D, EB], bf16, kind="ExternalInput")
    wv = nc.dram_tensor("wv", [D, EB], bf16, kind="ExternalInput")
    wo = nc.dram_tensor("wo", [D, D], bf16, kind="ExternalInput")
    bk = nc.dram_tensor("bk", [128, 8], f32, kind="ExternalInput")
    e64 = nc.dram_tensor("e64", [1, 65], bf16, kind="ExternalInput")
    mvn = nc.dram_tensor("mvn", [1, L], bf16, kind="ExternalInput")
    mv = nc.dram_tensor("mv", [1, L], f32, kind="ExternalInput")
    out = nc.dram_tensor("out", [D, 256], f32, kind="ExternalOutput")

    with tile.TileContext(nc) as tc:
        with (
            tc.tile_pool(name="sb", bufs=1) as sb,
            tc.tile_pool(name="rot", bufs=4) as rot,
            tc.tile_pool(name="ps_big", bufs=2, space="PSUM") as ps_big,
            tc.tile_pool(name="ps_at", bufs=4, space="PSUM") as ps_at,
            tc.tile_pool(name="dram", bufs=1, space="DRAM") as dram,
        ):
            # ---- constants / masks (tiny, first) ----
            bk_t = sb.tile([128, 8], f32, tag="bk")
            e64_t = sb.tile([1, 65], bf16, tag="e64")
            mvn_t = sb.tile([1, L], bf16, tag="mvn")
            mv_t = sb.tile([1, L], f32, tag="mv")
            nc.sync.dma_start(bk_t[:], bk[:])
            nc.sync.dma_start(e64_t[:], e64[:])
            nc.sync.dma_start(mvn_t[:], mvn[:])
            nc.sync.dma_start(mv_t[:], mv[:])
            # tiny dummy collective: warms ncfw + aligns cores early
            dumin = dram.tile([8, 32], f32, tag="dumin", name="dumin")
            dumout = dram.tile([64, 32], f32, tag="dumout", name="dumout")
            dum_sb = sb.tile([8, 32], f32, tag="dumsb")
            nc.gpsimd.memset(dum_sb[:], 1.0)
            nc.gpsimd.dma_start(dumin[:], dum_sb[:])
            nc.gpsimd.collective_compute(
                "AllGather", mybir.AluOpType.bypass,
                replica_groups=[list(range(NCORES))],
                ins=[dumin[:].opt()], outs=[dumout[:].opt()])

            # ---- input DMAs: 2-chunk pairs (halves the serialized issue
            # cost on the sequencers), split over SP+ACT queues ----
            def pair_load(eng, dram_t, F, i, tagp):
                t2 = sb.tile([128, 2 * F], bf16, tag=f"{tagp}{i}",
                             name=f"{tagp}{i}")
                dst = t2[:].rearrange("p (h t) -> p h t", h=2)
                srcv = dram_t[256 * i:256 * (i + 1), :].rearrange(
                    "(h p) t -> p h t", p=128)
                eng.dma_start(dst, srcv)
                return t2

            # warmup inputs first on the gpsimd queue (keep warmup early)
            warm_w = sb.tile([128, 128], bf16, tag="warmw")
            warm_x = sb.tile([128, 512], bf16, tag="warmx")
            nc.gpsimd.memset(warm_w[:], 0.5)
            nc.gpsimd.memset(warm_x[:], 0.5)
            qt_2, kt_2, wq_2, wk_2, wv_2 = [], [], [], [], []
            for i in range(4):
                wq_2.append(pair_load(nc.sync, wq, EB, i, "wq"))
                qt_2.append(pair_load(nc.sync, qt, L, i, "qt"))
                wk_2.append(pair_load(nc.scalar, wk, EB, i, "wk"))
                kt_2.append(pair_load(nc.scalar, kt, L, i, "kt"))
                wv_2.append(pair_load(nc.gpsimd, wv, EB, i, "wv"))
            mvbc = sb.tile([128, L], f32, tag="mvbc")
            nc.gpsimd.partition_broadcast(mvbc[:], mv_t[:])
            qt_t = [qt_2[i // 2][:, L * (i % 2):L * (i % 2 + 1)]
                    for i in range(8)]
            kt_t = [kt_2[i // 2][:, L * (i % 2):L * (i % 2 + 1)]
                    for i in range(8)]
            wq_t = [wq_2[i // 2][:, EB * (i % 2):EB * (i % 2 + 1)]
                    for i in range(8)]
            wk_t = [wk_2[i // 2][:, EB * (i % 2):EB * (i % 2 + 1)]
                    for i in range(8)]
            wv_t = [wv_2[i // 2][:, EB * (i % 2):EB * (i % 2 + 1)]
                    for i in range(8)]

            # ---- HAM warmup: dummy matmuls on garbage (no input deps) ----
            warm_ps = ps_at.tile([128, 512], f32, tag="at", name="warm_ps")
            for w in range(9):
                nc.tensor.matmul(warm_ps[:], warm_w[:], warm_x[:],
                                 start=(w == 0), stop=(w == 8))

            # ---- phase 1: projections per feature half (et0 first so the
            # first attention block starts as early as possible) ----
            qpt = [sb.tile([128, L], bf16, tag=f"qpt{i}", name=f"qpt{i}")
                   for i in range(2)]
            kpt = [sb.tile([128, L], bf16, tag=f"kpt{i}", name=f"kpt{i}")
                   for i in range(2)]

            def project(dst, w_t, x_t, et):
                # dc-innermost: lhsT changes every matmul so the PE can
                # prefetch the next LDWEIGHTS into the background buffer
                for qc in range(2):
                    pj = ps_big.tile([128, 512], f32, tag="big",
                                     name=f"pj{id(dst)}{et}{qc}")
                    for dc in range(8):
                        nc.tensor.matmul(
                            pj[:],
                            w_t[dc][:, 128 * et:128 * (et + 1)],
                            x_t[dc][:, 512 * qc:512 * (qc + 1)],
                            start=(dc == 0), stop=(dc == 7))
                    qs2 = slice(512 * qc, 512 * (qc + 1))
                    if dst is qpt:
                        # fold query-mask into Qp (residual uses masked Qp)
                        nc.vector.tensor_mul(dst[et][:, qs2], pj[:],
                                             mvbc[:, qs2])
                    else:
                        nc.vector.tensor_copy(dst[et][:, qs2], pj[:])

            project(qpt, wq_t, qt_t, 0)
            project(kpt, wk_t, kt_t, 0)
            project(qpt, wq_t, qt_t, 1)
            project(kpt, wk_t, kt_t, 1)

            # Vp natural [k-tokens, e] with ones column per head (65-stride)
            vpa = [sb.tile([128, 65 * HPC], bf16, tag=f"vpa{i}", name=f"vpa{i}")
                   for i in range(8)]
            for tt in range(8):
                nc.gpsimd.memset(vpa[tt][:], 1.0)

            def vproj(tt):
                pv = ps_at.tile([128, EB], f32, tag="at", name=f"pv{tt}")
                for dc in range(8):
                    nc.tensor.matmul(
                        pv[:], kt_t[dc][:, 128 * tt:128 * (tt + 1)], wv_t[dc][:],
                        start=(dc == 0), stop=(dc == 7))
                for h in range(HPC):
                    nc.vector.tensor_copy(
                        vpa[tt][:, 65 * h:65 * h + 64],
                        pv[:, 64 * h:64 * (h + 1)])

            vproj(0)
            vproj(1)

            # ---- epilogue weights (prefetch, after phase-1 loads) ----
            wo_t = [sb.tile([128, D], bf16, tag=f"wo{i}", name=f"wo{i}")
                    for i in range(8)]
            wo_dmas = []
            for i in range(8):
                wo_dmas.append(
                    nc.sync.dma_start(wo_t[i][:], wo[128 * i:128 * (i + 1), :]))

            # ---- phase 2+3: attention q-chunk-major; A2A per q-chunk ----
            # A2A chunk j (-> rank j): [256 e, 64 t] at tokens 512*qc + 64*j.
            inb = [dram.tile([2048, 64], bf16, tag=f"inb{i}", name=f"inb{i}")
                   for i in range(2)]
            outb = [dram.tile([2048, 64], bf16, tag=f"outb{i}", name=f"outb{i}")
                    for i in range(2)]
            attn = [sb.tile([128, L], bf16, tag=f"attn{i}", name=f"attn{i}")
                    for i in range(2)]
            ot = [sb.tile([128, L], bf16, tag=f"ot{i}", name=f"ot{i}")
                  for i in range(2)]
            # ot_sl[dt] columns: [qc0-b0 | qc0-b1 | qc1-b0 | qc1-b1], 64 each
            ot_sl = [sb.tile([128, 256], bf16, tag=f"osl{i}", name=f"osl{i}")
                     for i in range(8)]
            from concourse import mybir as _mb

            last_attn_mm = None
            for qc in range(2):
                qs = slice(512 * qc, 512 * (qc + 1))
                for hp in (0, 2):
                    et = hp // 2
                    ats = []
                    for h in (hp, hp + 1):
                        at = ps_at.tile([65, 512], f32, tag="at",
                                        name=f"at{h}_{qc}")
                        nc.tensor.matmul(at[:], e64_t[:], mvn_t[:, qs],
                                         start=True, stop=False)
                        ats.append(at)
                    # software pipeline: S(k) issued ahead of attn(k-1)
                    p_prev = None
                    for ki in range(8):
                        ks = slice(128 * ki, 128 * (ki + 1))
                        s_ps = ps_big.tile([128, 1024], f32, tag="big",
                                           name=f"s{hp}_{qc}_{ki}")
                        for j in range(2):
                            ro = 64 * j
                            nc.tensor.matmul(
                                s_ps[:, 512 * j:512 * (j + 1)],
                                kpt[et][ro:ro + 64, ks],
                                qpt[et][ro:ro + 64, qs],
                                start=True, stop=True)
                        p_t = rot.tile([128, 1024], bf16, tag="p",
                                       name=f"p{hp}_{qc}_{ki}")
                        act = nc.scalar.activation(p_t[:], s_ps[:], EXP,
                                                   bias=bk_t[:, ki:ki + 1],
                                                   scale=1.0 / 32)
                        if qc == 0 and hp == 0:
                            if ki == 0:
                                for dma in wo_dmas:
                                    add_dep_helper(dma.ins, act.ins, sync=True,
                                                   reason="defer wo to attn")
                            if ki < 6:
                                vproj(ki + 2)
                        if p_prev is not None:
                            kp_, pp = p_prev
                            for j, h in enumerate((hp, hp + 1)):
                                nc.tensor.matmul(
                                    ats[j][:], vpa[kp_][:, 65 * h:65 * h + 65],
                                    pp[:, 512 * j:512 * (j + 1)],
                                    start=False, stop=False)
                        p_prev = (ki, p_t)
                    kp_, pp = p_prev
                    for j, h in enumerate((hp, hp + 1)):
                        last_attn_mm = nc.tensor.matmul(
                            ats[j][:], vpa[kp_][:, 65 * h:65 * h + 65],
                            pp[:, 512 * j:512 * (j + 1)],
                            start=False, stop=True)
                    # normalize: attn = at[0:64] / denom  (denom row 64)
                    for j, h in enumerate((hp, hp + 1)):
                        ro = 64 * (h % 2)
                        den = rot.tile([1, 512], f32, tag="den")
                        nc.scalar.activation(den[:], ats[j][64:65, :],
                                             mybir.ActivationFunctionType.Copy)
                        rcp = rot.tile([1, 512], f32, tag="rcp")
                        nc.vector.reciprocal_approx_fast(rcp[:], den[:])
                        bc = rot.tile([64, 512], f32, tag="bc")
                        nc.gpsimd.partition_broadcast(bc[:], rcp[:])
                        nc.vector.tensor_mul(
                            attn[et][ro:ro + 64, qs], ats[j][0:64, :], bc[:])
                    # this feature half is complete: residual + bounce write
                    nc.vector.tensor_add(ot[et][:, qs], qpt[et][:, qs],
                                         attn[et][:, qs])
                    srcv2 = ot[et][:, qs].rearrange("p (j c) -> p j c", j=8)
                    dstv2 = inb[qc][:].rearrange("(j h p) c -> j h p c",
                                                 j=8, h=2)[:, et, :, :]
                    nc.gpsimd.dma_start(dstv2.rearrange("j p c -> p j c"),
                                        srcv2)
                # (residual + bounce writes are emitted per head pair above)
                nc.gpsimd.collective_compute(
                    "AllToAll", _mb.AluOpType.bypass,
                    replica_groups=[list(range(NCORES))],
                    ins=[inb[qc][:].opt()], outs=[outb[qc][:].opt()])

            # read back (after both triggers so SP FIFO never blocks a trigger)
            def readback(qc, dt_, eng):
                g, hh = dt_ // 2, dt_ % 2
                # rows (g b hh p): chunk j = 2g + b (core id = 2g + b)
                srcv = outb[qc][:].rearrange(
                    "(g b h p) c -> g b h p c", g=4, b=2, h=2)[g, :, hh, :, :]
                srcv = srcv.rearrange("b p c -> p b c")
                dst = ot_sl[dt_][:, 128 * qc:128 * (qc + 1)].rearrange(
                    "p (b c) -> p b c", b=2)
                eng.dma_start(dst, srcv)

            for dt_ in range(8):
                readback(0, dt_, nc.sync)

            # ---- phase 4: output projection epilogue (pass per q-chunk) ----
            prev_pass_mm = last_attn_mm
            for p in range(2):
                first_mms = []
                pass_last = None
                o_big = sb.tile([128, 1024], f32, tag=f"obig{p}", name=f"obig{p}")
                for et in range(8):
                    fpt = ps_big.tile([128, 128], f32, tag="big", name=f"fp{p}_{et}")
                    for dc in range(8):
                        mm = nc.tensor.matmul(
                            fpt[:], wo_t[dc][:, 128 * et:128 * (et + 1)],
                            ot_sl[dc][:, 128 * p:128 * (p + 1)],
                            start=(dc == 0), stop=(dc == 7))
                        if dc == 0:
                            first_mms.append(mm)
                        pass_last = mm
                    ff = rot.tile([128, 128], f32, tag="ff")
                    nc.scalar.activation(ff[:], fpt[:], RELU)
                    nc.vector.tensor_add(o_big[:, 128 * et:128 * (et + 1)],
                                         ff[:],
                                         ot_sl[et][:, 128 * p:128 * (p + 1)])
                # four strided DMAs per pass (each starts after 2 adds)
                for hf in range(4):
                    dstv = out[256 * hf:256 * (hf + 1),
                               128 * p:128 * (p + 1)].rearrange(
                        "(et p2) c -> p2 et c", p2=128)
                    srcv = o_big[:, 256 * hf:256 * (hf + 1)].rearrange(
                        "p2 (et c) -> p2 et c", et=2)
                    nc.sync.dma_start(dstv, srcv)
                if p == 0:
                    for dt_ in range(8):
                        readback(1, dt_, nc.scalar if dt_ % 2 else nc.sync)
                # pin PE order: this pass's first MMs after previous pass/attn
                for mm in first_mms:
                    add_dep_helper(mm.ins, prev_pass_mm.ins, sync=False,
                                   reason="epilogue pass ordering")
                prev_pass_mm = pass_last

    nc.compile()
    return nc


def _get_compiled():
    global _compiled
    if _compiled is None:
        _compiled = _build()
    return _compiled


def kernel(Q, K, mask_Q, mask_K, Wq, Wk, Wv, Wo):
    global LAST_RESULTS
    import ml_dtypes
    from concourse.bass_utils import run_bass_kernel_spmd

    bf = ml_dtypes.bfloat16
    Q = np.asarray(Q, np.float32)
    K = np.asarray(K, np.float32)
    mask_Q = np.asarray(mask_Q, bool)
    mask_K = np.asarray(mask_K, bool)
    Wq = np.asarray(Wq, np.float32)
    Wk = np.asarray(Wk, np.float32)
    Wv = np.asarray(Wv, np.float32)
    Wo = np.asarray(Wo, np.float32)

    nc = _get_compiled()

    e64v = np.zeros((1, 65), np.float32)
    e64v[0, 64] = BIG
    wot = np.ascontiguousarray(Wo.T.astype(bf))
    in_maps = []
    for c in range(NCORES):
        b, g = c % 2, c // 2
        eb = slice(EB * g, EB * (g + 1))
        bias = np.where(mask_K[b], NEG, 0.0).astype(np.float32)
        in_maps.append({
            "qt": np.ascontiguousarray(Q[b].T.astype(bf)),
            "kt": np.ascontiguousarray(K[b].T.astype(bf)),
            "wq": np.ascontiguousarray(Wq[eb, :].T.astype(bf)),
            "wk": np.ascontiguousarray(Wk[eb, :].T.astype(bf)),
            "wv": np.ascontiguousarray(Wv[eb, :].T.astype(bf)),
            "wo": wot,
            "bk": np.ascontiguousarray(bias.reshape(8, 128).T),
            "e64": e64v.astype(bf),
            "mvn": mask_Q[b].astype(bf)[None, :],
            "mv": (~mask_Q[b]).astype(np.float32)[None, :],
        })

    res = run_bass_kernel_spmd(nc, in_maps, core_ids=list(range(NCORES)),
                               trace=TRACE, **TRACE_KWARGS)
    LAST_RESULTS = res

    full = np.empty((B, L, D), np.float32)
    for c in range(NCORES):
        o = res.results[c]["out"]   # [1024 e, 256]: [qc0-b0|qc0-b1|qc1-b0|qc1-b1]
        full[0, 64 * c:64 * (c + 1), :] = o[:, 0:64].T
        full[1, 64 * c:64 * (c + 1), :] = o[:, 64:128].T
        full[0, 512 + 64 * c:512 + 64 * (c + 1), :] = o[:, 128:192].T
        full[1, 512 + 64 * c:512 + 64 * (c + 1), :] = o[:, 192:256].T
    return full

